# revision 1
# baseline (speedup 1.0000x reference)
"""Causal multi-head attention (B=4, N=2048, C=1024, H=16) on 8 Trainium2 cores.

Sharding: data-parallel over batch (4) x tensor-parallel over heads (2 groups
of 8).  Core c handles batch c//2, head-group c%2.  Each core computes its
heads' attention and a partial output projection; the host sums the two
head-group partials per batch and adds the bias.

Device layout notes (per core):
  - All matmul operands are bf16; accumulation fp32 in PSUM.
  - x, weights are shipped pre-transposed so QKV lands as q^T/k^T [d, n].
  - Scores are computed transposed (S^T[kv, q]) so softmax's exp feeds the
    PV matmul directly without transposing the probability matrix.
  - No max-subtraction in softmax: scores are O(1) (std ~1) by construction,
    exp never overflows fp32.  The causal mask is added via an
    identity-matmul of an additive mask tile into PSUM before the score
    matmul accumulates on top.
  - The softmax denominator comes for free from a 65th all-ones column
    appended to V (row 64 of the PV PSUM output).
  - Output projection consumes attn^T and produces out^T; the host
    transposes while unsharding.
"""

import numpy as np
import ml_dtypes

BF16 = ml_dtypes.bfloat16

B, N, C, H, D = 4, 2048, 1024, 16, 64
HPC = 8            # heads per core
GD = HPC * D       # 512 channels per head-group
P = 128
KC = C // P        # 8 contraction chunks for the projections
SPAN = 512         # query-column span processed per attention step
NSPAN = N // SPAN
NEG = -28672.0     # additive mask; exactly representable in bf16

_CACHE = {}


def _emit_once(tc, mybir, xT_d, wqkT_d, wvT_d, wpT_d, bm_d, id_d, out_d,
               phases):
    nc = tc.nc
    dt = mybir.dt
    f32, bf = dt.float32, dt.bfloat16
    Exp = mybir.ActivationFunctionType.Exp
    MUL = mybir.AluOpType.mult
    do_qkv = "qkv" in phases
    do_attn = "attn" in phases
    do_proj = "proj" in phases

    with (
        tc.tile_pool(name="weights", bufs=1) as wp,
        tc.tile_pool(name="acts", bufs=1) as ab,
        tc.tile_pool(name="small", bufs=4) as sp,
        tc.tile_pool(name="ps", bufs=1, space="PSUM") as ps,
        tc.tile_pool(name="aTp", bufs=2) as aTp,
        tc.tile_pool(name="exp", bufs=4) as exp_pool,
    ):
        # ---------------- input loads (chunked: DMA parallelism + fine deps)
        xk = [[wp.tile([P, N // 2], bf, tag=f"xk{k}_{h2}", name=f"xk{k}_{h2}")
               for h2 in range(2)] for k in range(KC)]
        wqk = [wp.tile([P, 2 * GD], bf, tag=f"wqk{k}", name=f"wqk{k}")
               for k in range(KC)]
        wv = [wp.tile([P, GD], bf, tag=f"wv{k}", name=f"wv{k}")
              for k in range(KC)]
        for k in range(KC):
            for h2 in range(2):
                nc.sync.dma_start(
                    xk[k][h2],
                    xT_d[k * P:(k + 1) * P,
                         h2 * (N // 2):(h2 + 1) * (N // 2)])
            nc.sync.dma_start(wqk[k], wqkT_d[k * P:(k + 1) * P, :])
            nc.sync.dma_start(wv[k], wvT_d[k * P:(k + 1) * P, :])
        wpk = [wp.tile([P, C], bf, tag=f"wpk{k}", name=f"wpk{k}")
               for k in range(GD // P)]
        for k in range(GD // P):
            nc.sync.dma_start(wpk[k], wpT_d[k * P:(k + 1) * P, :])
        bm = wp.tile([P, 2 * SPAN], bf, tag="bm")
        nc.sync.dma_start(bm, bm_d)
        i128 = wp.tile([P, P], bf, tag="i128")
        nc.sync.dma_start(i128, id_d)

        # q^T/k^T rows: per (128-row chunk, 512-col quarter) tiles so
        # attention can start before a chunk's later columns are computed
        qkm = [[ab.tile([P, SPAN], bf, tag=f"qkm{m}_{q}", name=f"qkm{m}_{q}")
                for q in range(4)] for m in range(2 * GD // P)]
        # V per kv-block with an all-ones 65th column per head
        vab = [ab.tile([P, HPC * (D + 1)], bf, tag=f"vab{m}", name=f"vab{m}")
               for m in range(N // P)]

        # PSUM bank budget (8 banks of [128, 512]f32):
        #   qk (QKV groups)     [128, 2, 512] x1  = 2
        #   duo/pp              [128, 2, 512] x2  = 4
        #   oA, oB              [65, 512]     x1  = 2
        def qk_chunk(m):
            if not do_qkv:
                return
            for q in range(4):
                pg = ps.tile([P, SPAN], f32, tag="qk", name=f"pg{m}{q}",
                             bufs=2)
                for k in range(KC):
                    nc.tensor.matmul(
                        pg,
                        wqk[k][:, m * P:(m + 1) * P],
                        xk[k][q // 2][:, (q % 2) * SPAN:(q % 2 + 1) * SPAN],
                        start=(k == 0),
                        stop=(k == KC - 1),
                    )
                nc.vector.tensor_copy(out=qkm[m][q], in_=pg)

        def v_chunk(m4):
            if not do_qkv:
                return
            for sub in range(4):
                m16 = m4 * 4 + sub
                pv = ps.tile([P, SPAN], f32, tag="qk", name=f"pv{m16}",
                             bufs=2)
                nc.vector.memset(vab[m16], 1.0)
                for k in range(KC):
                    nc.tensor.matmul(
                        pv,
                        xk[k][m16 // 8][:, (m16 % 8) * P:(m16 % 8 + 1) * P],
                        wv[k],
                        start=(k == 0),
                        stop=(k == KC - 1),
                    )
                nc.vector.tensor_copy(
                    out=vab[m16].rearrange(
                        "p (h e) -> p h e", h=HPC)[:, :, :D],
                    in_=pv.rearrange("p (h e) -> p h e", h=HPC),
                )

        def attn_pair(J, hp, acT):
            if not do_attn:
                return
            nblk = 4 * (J + 1)
            qs = J * SPAN
            outs = (
                ps.tile([65, SPAN], f32, tag="oA", name="oA", bufs=1),
                ps.tile([65, SPAN], f32, tag="oB", name="oB", bufs=1),
            )
            def emit_pv(ex, j2, lo):
                for hi in (0, 1):
                    h = 2 * hp + hi
                    nc.tensor.matmul(
                        outs[hi][:, lo:],
                        vab[j2][:, h * (D + 1):(h + 1) * (D + 1)],
                        ex[:, hi, lo:],
                        start=(j2 == 0),
                        stop=(j2 == nblk - 1),
                    )

            pend = None  # software pipeline: PV one block behind scores/exp
            for j2 in range(nblk):
                duo = ps.tile([P, 2, SPAN], f32, tag="duo", bufs=2)
                dtg = j2 - 4 * J   # >=0: diagonal block index
                lo = P * dtg if dtg >= 0 else 0  # first live column
                diag = dtg >= 0
                if diag:
                    # triangle masks for both heads first, so the two score
                    # matmuls issue back-to-back and row-pack concurrently
                    for hi in (0, 1):
                        nc.tensor.matmul(
                            duo[:, hi, lo:lo + P], i128,
                            bm[:, SPAN:SPAN + P],
                            start=True, stop=False,
                        )
                for hi in (0, 1):
                    nc.tensor.matmul(
                        duo[:, hi, lo:],
                        qkm[4 + hp][j2 // 4][64 * hi:64 * (hi + 1),
                                             (j2 % 4) * P:(j2 % 4 + 1) * P],
                        qkm[hp][J][64 * hi:64 * (hi + 1), lo:],
                        start=not diag,
                        stop=True,
                    )
                ex = exp_pool.tile([P, 2, SPAN], bf, tag="ex")
                nc.scalar.activation(ex[:, :, lo:], duo[:, :, lo:], Exp)
                if pend is not None:
                    emit_pv(*pend)
                pend = (ex, j2, lo)
            emit_pv(*pend)
            for hi in (0, 1):
                o = outs[hi]
                rc = sp.tile([1, SPAN], f32, tag="rc")
                nc.vector.reciprocal(rc, o[64:65, :])
                bc = sp.tile([64, SPAN], f32, tag="bc")
                nc.gpsimd.partition_broadcast(bc, rc)
                nc.vector.tensor_tensor(
                    acT[64 * hi:64 * (hi + 1), hp, :], o[0:64, :], bc, MUL,
                )

        def proj_span(J, acT):
            if not do_proj:
                return
            qs = J * SPAN
            for mo in range(C // P):
                pp = ps.tile([P, SPAN], f32, tag="duo", name=f"pp{mo}",
                             bufs=2)
                for k in range(GD // P):
                    nc.tensor.matmul(
                        pp,
                        wpk[k][:, mo * P:(mo + 1) * P],
                        acT[:, k, :],
                        start=(k == 0),
                        stop=(k == GD // P - 1),
                    )
                ob = sp.tile([P, SPAN], f32, tag="ob")
                nc.vector.tensor_copy(out=ob, in_=pp)
                nc.sync.dma_start(out_d[mo * P:(mo + 1) * P, qs:qs + SPAN],
                                  ob)

        # Interleaved emission: attention (span J, pair hp) needs qkm[hp],
        # qkm[4+hp], vab[0..4J+3]; unblock hp pairs of span 0 early so ACT
        # overlaps the QKV phase.
        acTs = [aTp.tile([P, GD // P, SPAN], bf, tag="acT", name=f"acT{J}")
                for J in range(NSPAN)]
        qk_chunk(0)
        qk_chunk(4)
        v_chunk(0)
        attn_pair(0, 0, acTs[0])
        qk_chunk(1)
        qk_chunk(5)
        attn_pair(0, 1, acTs[0])
        qk_chunk(2)
        qk_chunk(6)
        attn_pair(0, 2, acTs[0])
        qk_chunk(3)
        qk_chunk(7)
        attn_pair(0, 3, acTs[0])
        v_chunk(1)
        attn_pair(1, 0, acTs[1])
        proj_span(0, acTs[0])
        for hp in range(1, 4):
            attn_pair(1, hp, acTs[1])
        v_chunk(2)
        attn_pair(2, 0, acTs[2])
        proj_span(1, acTs[1])
        for hp in range(1, 4):
            attn_pair(2, hp, acTs[2])
        v_chunk(3)
        attn_pair(3, 0, acTs[3])
        proj_span(2, acTs[2])
        for hp in range(1, 4):
            attn_pair(3, hp, acTs[3])
        proj_span(3, acTs[3])


def _emit(tc, mybir, reps=1, phases=("qkv", "attn", "proj")):
    nc = tc.nc
    dt = mybir.dt
    f32, bf = dt.float32, dt.bfloat16

    xT_d = nc.dram_tensor("xT", [C, N], bf, kind="ExternalInput").ap()
    wqkT_d = nc.dram_tensor("wqkT", [C, 2 * GD], bf, kind="ExternalInput").ap()
    wvT_d = nc.dram_tensor("wvT", [C, GD], bf, kind="ExternalInput").ap()
    wpT_d = nc.dram_tensor("wpT", [GD, C], bf, kind="ExternalInput").ap()
    bm_d = nc.dram_tensor("BM", [P, 2 * SPAN], bf, kind="ExternalInput").ap()
    id_d = nc.dram_tensor("I128", [P, P], bf, kind="ExternalInput").ap()
    out_d = nc.dram_tensor("outT", [C, N], f32, kind="ExternalOutput").ap()

    for _rep in range(reps):
        _emit_once(tc, mybir, xT_d, wqkT_d, wvT_d, wpT_d, bm_d, id_d, out_d,
                   phases)


def _get_module(reps=1, phases=("qkv", "attn", "proj")):
    key = (reps, tuple(phases))
    if key not in _CACHE:
        import concourse.tile as tile
        from concourse import bacc, mybir

        nc = bacc.Bacc("TRN2", target_bir_lowering=False, debug=False,
                       num_devices=8)
        with tile.TileContext(nc) as tc:
            _emit(tc, mybir, reps=reps, phases=phases)
        nc.compile()
        _CACHE[key] = nc
    return _CACHE[key]


def _host_inputs(x, w_qkv, w_proj):
    scale = D ** -0.5
    bmask = np.full((P, 2 * SPAN), NEG, np.float32)
    for p in range(P):
        bmask[p, p + SPAN:] = 0.0
    bmask = bmask.astype(BF16)
    ident = np.eye(P, dtype=BF16)
    in_maps = []
    for core in range(8):
        b, g = core // 2, core % 2
        rows = slice(g * GD, (g + 1) * GD)
        wq = w_qkv[0 * C:1 * C][rows] * scale
        wk = w_qkv[1 * C:2 * C][rows]
        wv = w_qkv[2 * C:3 * C][rows]
        in_maps.append({
            "xT": np.ascontiguousarray(x[b].T).astype(BF16),
            "wqkT": np.ascontiguousarray(
                np.concatenate([wq, wk], axis=0).T).astype(BF16),
            "wvT": np.ascontiguousarray(wv.T).astype(BF16),
            "wpT": np.ascontiguousarray(w_proj[:, rows].T).astype(BF16),
            "BM": bmask,
            "I128": ident,
        })
    return in_maps


def kernel(x, w_qkv, w_proj, b_proj, _trace=False):
    from concourse.bass_utils import run_bass_kernel_spmd

    nc = _get_module()
    in_maps = _host_inputs(np.asarray(x, np.float32),
                           np.asarray(w_qkv, np.float32),
                           np.asarray(w_proj, np.float32))
    res = run_bass_kernel_spmd(nc, in_maps, core_ids=list(range(8)),
                               trace=_trace)
    outs = [r["outT"] for r in res.results]
    out = np.empty((B, N, C), np.float32)
    bp = np.asarray(b_proj, np.float32)[None, :]
    for b in range(B):
        out[b] = outs[2 * b].T + outs[2 * b + 1].T + bp
    if _trace:
        kernel._last_results = res
    return out



# revision 3
# speedup vs baseline: 1.2150x; 1.2150x over previous
"""Causal multi-head attention (B=4, N=2048, C=1024, H=16) on 8 Trainium2 cores.

Sharding: data-parallel over batch (4) x tensor-parallel over heads (2 groups
of 8).  Core c handles batch c//2, head-group c%2.  Each core computes its
heads' attention and a partial output projection; the host sums the two
head-group partials per batch and adds the bias.

Device design (per core):
  - All matmul operands bf16; fp32 accumulation in PSUM.
  - Scores computed transposed (S^T[kv, q]); exp of the raw scores (inputs
    are O(1) by construction so exp never overflows); the causal triangle of
    the diagonal 128-blocks is zeroed AFTER the exp by a gpsimd multiply
    with a 0/1 triangle tile (no PE mask matmuls, no -inf handling).
  - PV is "flipped": the exp'd probability chunk [128kv, 128q] is the
    stationary operand and V [128kv, 65] is the moving one (64 value columns
    plus an all-ones column that accumulates the softmax denominator), so a
    PV matmul costs 65 PE rows instead of 512 and fully-masked chunks are
    skipped.  Output lands as attn[q, head*64+d] per 128-query chunk.
  - Per (head-pair): two single-bank PSUM tiles (hi=0/hi=1) hold the four
    query-chunk accumulators [128, 4, 65].  Exactly one start=True matmul
    per tile arms the bank's lazy-zero region; every other chain relies on
    first-touch zeroing (PSUM zero regions are 2KiB = 1 bank).
  - Normalization: DVE reciprocal of the denominator column, then per-head
    tensor_scalar multiplies into bf16 attn chunks [128q, 512gd].
  - attn -> attn^T for the projection via DMA xbar transposes (idle DMA
    engines), writing acT [128gd, 4k, 4c, 128q] consumed by the projection.
  - Emission-order scheduler: the attention stream is Act-paced (exp), so
    QKV / projection / PV work is woven between score blocks to keep the PE
    busy; a deque of filler closures is popped on a cycle-estimate pacer,
    with forced pops for data dependencies.
"""

import numpy as np
import ml_dtypes
from collections import deque

BF16 = ml_dtypes.bfloat16

B, N, C, H, D = 4, 2048, 1024, 16, 64
HPC = 8            # heads per core
GD = HPC * D       # 512 channels per head-group
P = 128
KC = C // P        # 8 contraction chunks for the QKV projection
SPAN = 512         # query span per attention step
NSPAN = N // SPAN
PPS = 4            # head pairs per core

_CACHE = {}


def _emit_once(tc, mybir, xT_d, wqkT_d, wvT_d, wpT_d, tri_d, out_d):
    nc = tc.nc
    dt = mybir.dt
    f32, bf = dt.float32, dt.bfloat16
    Exp = mybir.ActivationFunctionType.Exp
    MUL = mybir.AluOpType.mult

    with (
        tc.tile_pool(name="weights", bufs=1) as wp,
        tc.tile_pool(name="acts", bufs=1) as ab,
        tc.tile_pool(name="exp", bufs=6) as exp_pool,
        tc.tile_pool(name="attn", bufs=1) as atp,
        tc.tile_pool(name="small", bufs=4) as sp,
    ):
        # ---------------- persistent SBUF tiles
        xq = [[wp.tile([P, SPAN], bf, tag=f"xq{k}_{q}", name=f"xq{k}_{q}")
               for q in range(4)] for k in range(KC)]
        wqk = [wp.tile([P, 2 * GD], bf, tag=f"wqk{k}", name=f"wqk{k}")
               for k in range(KC)]
        wv = [wp.tile([P, GD], bf, tag=f"wv{k}", name=f"wv{k}")
              for k in range(KC)]
        wpk = [wp.tile([P, C], bf, tag=f"wpk{k}", name=f"wpk{k}")
               for k in range(GD // P)]
        tri = wp.tile([P, 2, P], bf, tag="tri")
        qkm = [[ab.tile([P, SPAN], bf, tag=f"qkm{m}_{q}", name=f"qkm{m}_{q}")
                for q in range(4)] for m in range(2 * GD // P)]
        vab = [ab.tile([P, HPC, D + 1], bf, tag=f"vab{m}", name=f"vab{m}")
               for m in range(N // P)]

        # ---------------- DMA issue order: first-needed first
        for k in range(KC):
            nc.sync.dma_start(wqk[k], wqkT_d[k * P:(k + 1) * P, :])
            nc.sync.dma_start(xq[k][0], xT_d[k * P:(k + 1) * P, 0:SPAN])
        for k in range(KC):
            nc.sync.dma_start(wv[k], wvT_d[k * P:(k + 1) * P, :])
        nc.sync.dma_start(tri, tri_d)
        for k in range(KC):
            nc.sync.dma_start(xq[k][1], xT_d[k * P:(k + 1) * P, SPAN:2 * SPAN])
        for k in range(GD // P):
            nc.sync.dma_start(wpk[k], wpT_d[k * P:(k + 1) * P, :])
        for q in (2, 3):
            for k in range(KC):
                nc.sync.dma_start(xq[k][q],
                                  xT_d[k * P:(k + 1) * P,
                                       q * SPAN:(q + 1) * SPAN])
        for m16 in range(N // P):
            nc.vector.memset(vab[m16][:, :, D:], 1.0)

        # ---------------- emission-time pacing state
        st = {"pe": 0.0, "act": 0.0}

        def pe_add(cycles, n_inst=1):
            st["pe"] += cycles / 2.4 + n_inst * 5.0

        def act_add(ns):
            st["act"] += ns

        # ---------------- startup: quarter 0, k-major across all 8 qk chunks
        with tc.tile_pool(name="boot", bufs=8, space="PSUM") as bp:
            pg = [bp.tile([P, SPAN], f32, tag="boot", name=f"boot{m}")
                  for m in range(8)]
            for k in range(KC):
                for m in range(8):
                    nc.tensor.matmul(
                        pg[m], wqk[k][:, m * P:(m + 1) * P], xq[k][0],
                        start=(k == 0), stop=(k == KC - 1))
            for m in range(8):
                nc.scalar.copy(out=qkm[m][0], in_=pg[m])
                pe_add(8 * SPAN, 9)
                act_add(SPAN * 0.8333 + 300)
            for m16 in range(4):
                pv = bp.tile([P, GD], f32, tag="boot", name=f"bootv{m16}")
                for k in range(KC):
                    nc.tensor.matmul(
                        pv, xq[k][0][:, m16 * P:(m16 + 1) * P], wv[k],
                        start=(k == 0), stop=(k == KC - 1))
                nc.vector.tensor_copy(
                    out=vab[m16][:, :, :D],
                    in_=pv.rearrange("p (h e) -> p h e", h=HPC))
                pe_add(8 * GD, 9)

        # ---------------- filler machinery
        emitted = set()
        fillers = deque()

        def emit_qk(ps, m, q):
            pgt = ps.tile([P, SPAN], f32, tag="fill", bufs=2,
                          name=f"qk{m}_{q}")
            for k in range(KC):
                nc.tensor.matmul(
                    pgt, wqk[k][:, m * P:(m + 1) * P], xq[k][q],
                    start=(k == 0), stop=(k == KC - 1))
            nc.scalar.copy(out=qkm[m][q], in_=pgt)
            pe_add(8 * SPAN, 9)
            act_add(SPAN * 0.8333 + 300)

        def emit_v(ps, m16):
            q, t = divmod(m16, 4)
            pvt = ps.tile([P, GD], f32, tag="fill", bufs=2, name=f"v{m16}")
            for k in range(KC):
                nc.tensor.matmul(
                    pvt, xq[k][q][:, t * P:(t + 1) * P], wv[k],
                    start=(k == 0), stop=(k == KC - 1))
            nc.vector.tensor_copy(
                out=vab[m16][:, :, :D],
                in_=pvt.rearrange("p (h e) -> p h e", h=HPC))
            pe_add(8 * GD, 9)

        def emit_proj(ps, J, mo, acT):
            pp = ps.tile([P, SPAN], f32, tag="fill", bufs=2,
                         name=f"pp{J}_{mo}")
            for k in range(GD // P):
                nc.tensor.matmul(
                    pp, wpk[k][:, mo * P:(mo + 1) * P], acT[:, k, :, :],
                    start=(k == 0), stop=(k == GD // P - 1))
            ob = sp.tile([P, SPAN], f32, tag="ob")
            nc.vector.tensor_copy(out=ob, in_=pp)
            nc.sync.dma_start(
                out_d[mo * P:(mo + 1) * P, J * SPAN:(J + 1) * SPAN], ob)
            pe_add(4 * SPAN, 5)

        def run_filler(ps, item):
            kind = item[0]
            if kind == "qk":
                emit_qk(ps, item[1], item[2])
            elif kind == "v":
                emit_v(ps, item[1])
            else:
                emit_proj(ps, item[1], item[2], item[3])
            emitted.add(item[:3] if kind == "proj" else item)

        def pump(ps, lead=1500.0):
            while fillers and st["pe"] < st["act"] + lead:
                run_filler(ps, fillers.popleft())

        def need(ps, key):
            while key not in emitted:
                assert fillers, f"dependency {key} not in filler queue"
                run_filler(ps, fillers.popleft())

        for q in (1, 2, 3):
            for hp in range(PPS):
                fillers.append(("qk", hp, q))
                fillers.append(("qk", 4 + hp, q))
            for t in range(4):
                fillers.append(("v", 4 * q + t))
        for m16 in range(4):
            emitted.add(("v", m16))

        # ---------------- main attention loop
        with tc.tile_pool(name="ps", bufs=1, space="PSUM") as ps:
            acTs = []
            for J in range(NSPAN):
                nblk = 4 * (J + 1)
                attn_c = [atp.tile([P, SPAN], bf, tag=f"attn{c}", bufs=2,
                                   name=f"attn{J}_{c}") for c in range(4)]
                acT = atp.tile([P, 4, 4, P], bf, tag="acT", bufs=2,
                               name=f"acT{J}")
                for hp in range(PPS):
                    pvt = [ps.tile([P, 4, D + 1], f32, tag=f"pv{hi}", bufs=1,
                                   name=f"pv{J}_{hp}_{hi}") for hi in (0, 1)]
                    armed = [False, False]
                    exs = [None] * nblk
                    pvdone = 0

                    def emit_pv_block(j2):
                        ex, lo, dtg = exs[j2]
                        for c in range(max(dtg, 0), 4):
                            for hi in (0, 1):
                                nc.tensor.matmul(
                                    pvt[hi][:, c, :],
                                    ex[:, hi, c * P:(c + 1) * P],
                                    vab[j2][:, 2 * hp + hi, :],
                                    start=not armed[hi],
                                    stop=(j2 == 4 * J + c),
                                    skip_group_check=True,
                                )
                                armed[hi] = True
                        pe_add((4 - max(dtg, 0)) * 2 * (D + 1),
                               (4 - max(dtg, 0)) * 2)

                    if J > 0:
                        need(ps, ("qk", hp, J))
                        need(ps, ("qk", 4 + hp, J))
                    for j2 in range(nblk):
                        dtg = j2 - 4 * J
                        lo = P * dtg if dtg >= 0 else 0
                        duo = ps.tile([P, 2, SPAN], f32, tag="duo", bufs=2)
                        for hi in (0, 1):
                            nc.tensor.matmul(
                                duo[:, hi, lo:],
                                qkm[4 + hp][j2 // 4][
                                    64 * hi:64 * (hi + 1),
                                    (j2 % 4) * P:(j2 % 4 + 1) * P],
                                qkm[hp][J][64 * hi:64 * (hi + 1), lo:],
                                start=True, stop=True)
                        pe_add(2 * (SPAN - lo), 2)
                        ex = exp_pool.tile([P, 2, SPAN], bf, tag="ex")
                        nc.scalar.activation(ex[:, :, lo:], duo[:, :, lo:],
                                             Exp)
                        act_add(2 * (SPAN - lo) * 0.8333 + 300)
                        if dtg >= 0:
                            nc.gpsimd.tensor_tensor(
                                ex[:, :, lo:lo + P], ex[:, :, lo:lo + P],
                                tri, MUL)
                        exs[j2] = (ex, lo, dtg)
                        while pvdone < j2:
                            if ("v", pvdone) not in emitted:
                                need(ps, ("v", pvdone))
                            emit_pv_block(pvdone)
                            pvdone += 1
                        pump(ps)
                    while pvdone < nblk:
                        need(ps, ("v", pvdone))
                        emit_pv_block(pvdone)
                        pvdone += 1
                    # normalization for this pair's two heads
                    rc = [sp.tile([P, 4], f32, tag=f"rc{hi}", bufs=2,
                                  name=f"rc{J}_{hp}_{hi}") for hi in (0, 1)]
                    for hi in (0, 1):
                        nc.vector.reciprocal(rc[hi], pvt[hi][:, :, D])
                    for c in range(4):
                        for hi in (0, 1):
                            h = 2 * hp + hi
                            nc.vector.tensor_scalar_mul(
                                attn_c[c][:, h * D:(h + 1) * D],
                                pvt[hi][:, c, :D],
                                rc[hi][:, c:c + 1])
                # span end: transpose attn -> acT via DMA xbar
                for c in range(4):
                    nc.sync.dma_start_transpose(acT[:, :, c, :], attn_c[c])
                acTs.append(acT)
                for mo in range(C // P):
                    fillers.append(("proj", J, mo, acT))
            # tail: drain remaining fillers (late QKV leftovers + projections)
            while fillers:
                run_filler(ps, fillers.popleft())


def _emit(tc, mybir, reps=1, phases=None):
    nc = tc.nc
    dt = mybir.dt
    f32, bf = dt.float32, dt.bfloat16

    xT_d = nc.dram_tensor("xT", [C, N], bf, kind="ExternalInput").ap()
    wqkT_d = nc.dram_tensor("wqkT", [C, 2 * GD], bf, kind="ExternalInput").ap()
    wvT_d = nc.dram_tensor("wvT", [C, GD], bf, kind="ExternalInput").ap()
    wpT_d = nc.dram_tensor("wpT", [GD, C], bf, kind="ExternalInput").ap()
    tri_d = nc.dram_tensor("TRI", [P, 2, P], bf, kind="ExternalInput").ap()
    out_d = nc.dram_tensor("outT", [C, N], f32, kind="ExternalOutput").ap()

    for _rep in range(reps):
        _emit_once(tc, mybir, xT_d, wqkT_d, wvT_d, wpT_d, tri_d, out_d)


def _get_module(reps=1, phases=None):
    key = (reps,)
    if key not in _CACHE:
        import concourse.tile as tile
        from concourse import bacc, mybir

        nc = bacc.Bacc("TRN2", target_bir_lowering=False, debug=False,
                       num_devices=8)
        with tile.TileContext(nc) as tc:
            _emit(tc, mybir, reps=reps)
        nc.compile()
        _CACHE[key] = nc
    return _CACHE[key]


def _host_inputs(x, w_qkv, w_proj):
    scale = D ** -0.5
    tri01 = np.zeros((P, 2, P), np.float32)
    for kv in range(P):
        tri01[kv, :, kv:] = 1.0
    tri01 = tri01.astype(BF16)
    in_maps = []
    for core in range(8):
        b, g = core // 2, core % 2
        rows = slice(g * GD, (g + 1) * GD)
        wq = w_qkv[0 * C:1 * C][rows] * scale
        wk = w_qkv[1 * C:2 * C][rows]
        wv = w_qkv[2 * C:3 * C][rows]
        in_maps.append({
            "xT": np.ascontiguousarray(x[b].T).astype(BF16),
            "wqkT": np.ascontiguousarray(
                np.concatenate([wq, wk], axis=0).T).astype(BF16),
            "wvT": np.ascontiguousarray(wv.T).astype(BF16),
            "wpT": np.ascontiguousarray(w_proj[:, rows].T).astype(BF16),
            "TRI": tri01,
        })
    return in_maps


def kernel(x, w_qkv, w_proj, b_proj, _trace=False):
    from concourse.bass_utils import run_bass_kernel_spmd

    nc = _get_module()
    in_maps = _host_inputs(np.asarray(x, np.float32),
                           np.asarray(w_qkv, np.float32),
                           np.asarray(w_proj, np.float32))
    res = run_bass_kernel_spmd(nc, in_maps, core_ids=list(range(8)),
                               trace=_trace)
    outs = [r["outT"] for r in res.results]
    out = np.empty((B, N, C), np.float32)
    bp = np.asarray(b_proj, np.float32)[None, :]
    for b in range(B):
        out[b] = outs[2 * b].T + outs[2 * b + 1].T + bp
    if _trace:
        kernel._last_results = res
    return out
